# revision 7
# baseline (speedup 1.0000x reference)
"""Trainium2 Bass kernel for DTW features (open-end weighted DTW).

Problem: x (64, 6, 2048) f32, patts (64, 32) f32, w scalar.
  c[i,j]   = (patts[n,i] - x[b,d,j])^2
  D[0,j]   = c[0,j]
  D[i,j]   = c[i,j] + w * min(D[i-1,j], D[i,j-1], D[i-1,j-1])
  out[b,n,d,j] = sqrt(D[L-1,j])

Strategy: data-parallel over batch (8 batches per core).  Per (b, n, d)
tuple the DP runs row-by-row in the scaled domain Dt[i,j] = D[i,j]*w^-j,
which turns the recurrence into a hardware min/add scan along j:

  ct[i,j]  = c[i,j] * w^-j
  u[j]     = min(w*Dt[i-1,j], Dt[i-1,j-1])          (scalar_tensor_tensor)
  Dt[i,j]  = min(u[j], Dt[i,j-1]) + ct[i,j]         (tensor_tensor_scan)
  out      = sqrt(Dt[L-1,j] * w^j)

The cost matrix ct is produced by the TensorEngine as a rank-6 matmul:
ct[(s,n), j] = [p^2, -2p, 1] . [w^-j, x_s*w^-j, x_s^2*w^-j] with two
sequences s packed per 128-partition block (64 patterns each half).
"""

import os
import sys

import numpy as np

for _p in ("/opt/trn_rl_repo", "/root/.axon_site/_ro/trn_rl_repo"):
    if _p not in sys.path and os.path.isdir(_p):
        sys.path.insert(0, _p)

B, N, D, L, T = 64, 64, 6, 32, 2048
NCORES = 8
BLOC = B // NCORES            # batches per core
NSEQ = BLOC * D               # (b, d) sequences per core
NBLK = NSEQ // 2              # two sequences per 128-partition block
P, HALF = 128, 64
BIG = 1.0e30

_cache = {}

USE_V2 = True          # (i+j) scaling: ScalarE shl + DVE TT-min u (f32)
MAT_F32R = False       # float32r cost matmuls (4x faster PE, ~1e-3 cost err)
INTERLEAVE = 2         # blocks emitted round-robin in groups of this size


def _build(nblk, l_patts, t_len, w):
    """Build + compile the per-core Bass program (SPMD across 8 cores)."""
    import concourse.bacc as bacc
    import concourse.bass as bass
    import concourse.mybir as mybir
    import concourse.tile as tile

    f32 = mybir.dt.float32
    Alu = mybir.AluOpType
    Act = mybir.ActivationFunctionType
    CHUNK = min(512, t_len)
    nchunk = t_len // CHUNK

    nc = bacc.Bacc("TRN2", target_bir_lowering=False, debug=False,
                   num_devices=NCORES)

    mat_dt = mybir.dt.float32r if MAT_F32R else f32
    row_dt = f32

    rhs_d = nc.dram_tensor("rhs", [nblk, 6, t_len], mat_dt, kind="ExternalInput")
    lhsT_d = nc.dram_tensor("lhsT", [6, l_patts * P], mat_dt, kind="ExternalInput")
    wj_d = nc.dram_tensor("wj", [P, t_len], f32, kind="ExternalInput")
    out_d = nc.dram_tensor("out", [nblk, P, t_len], f32, kind="ExternalOutput")

    with tile.TileContext(nc) as tc:
        with (
            tc.tile_pool(name="const", bufs=1) as cpool,
            tc.tile_pool(name="rhs", bufs=2) as rpool,
            tc.tile_pool(name="rows", bufs=2) as dpool,
            tc.tile_pool(name="work", bufs=1) as wpool,
            tc.tile_pool(name="outp", bufs=2) as opool,
            tc.tile_pool(name="psum", bufs=2, space=bass.MemorySpace.PSUM) as ppool,
        ):
            lhsT_sb = cpool.tile([6, l_patts * P], mat_dt)
            nc.sync.dma_start(lhsT_sb[:], lhsT_d[:])
            wj_sb = cpool.tile([P, t_len], f32)
            nc.sync.dma_start(wj_sb[:], wj_d[:])

            def emit_matmuls(ct, rhs_sb, i):
                for k in range(nchunk):
                    nc.tensor.matmul(
                        ct[:, k * CHUNK:(k + 1) * CHUNK],
                        lhsT_sb[:, i * P:(i + 1) * P],
                        rhs_sb[:, k * CHUNK:(k + 1) * CHUNK],
                        start=True, stop=True,
                    )

            def emit_row_v1(st, i):
                ct = ppool.tile([P, t_len], f32, tag="ct")
                emit_matmuls(ct, st["rhs"], i)
                cur = st["rows"][i % 2]
                if i == 0:
                    nc.scalar.activation(cur[:, 1:t_len + 1], ct[:], Act.Copy)
                else:
                    prev = st["rows"][(i - 1) % 2]
                    u = wpool.tile([P, t_len], f32, tag=f"u{st['lane']}")
                    nc.vector.scalar_tensor_tensor(
                        u[:], prev[:, 1:t_len + 1], w, prev[:, 0:t_len],
                        Alu.mult, Alu.min,
                    )
                    nc.vector.tensor_tensor_scan(
                        cur[:, 1:t_len + 1], u[:], ct[:], BIG,
                        Alu.min, Alu.add,
                    )

            def emit_row_v2(st, i):
                # row tile layout: [P, t_len+2]; col 1 = BIG boundary (the
                # j=-1 cell), data in cols 2..t_len+1 (4B-aligned for bf16).
                # u[j] = min(Dp[j], Dp[j-1]/w): the shifted scaled read is
                # done by ScalarE (Copy, scale=1/w), so the DVE min is an
                # aligned bf16 tensor_tensor at 2x.
                ct = ppool.tile([P, t_len], f32, tag="ct")
                emit_matmuls(ct, st["rhs"], i)
                cur = st["rows"][i % 2]
                if i == 0:
                    nc.scalar.activation(cur[:, 2:t_len + 2], ct[:], Act.Copy)
                else:
                    prev = st["rows"][(i - 1) % 2]
                    shl = wpool.tile([P, t_len], row_dt, tag=f"shl{st['lane']}")
                    nc.scalar.activation(shl[:], prev[:, 1:t_len + 1],
                                         Act.Copy, scale=1.0 / w)
                    u = wpool.tile([P, t_len], row_dt, tag=f"u{st['lane']}")
                    nc.vector.tensor_tensor(
                        u[:], prev[:, 2:t_len + 2], shl[:], Alu.min)
                    nc.vector.tensor_tensor_scan(
                        cur[:, 2:t_len + 2], u[:], ct[:], BIG, Alu.min, Alu.add)

            G = max(1, INTERLEAVE)
            for g0 in range(0, nblk, G):
                lanes = []
                for blk in range(g0, min(g0 + G, nblk)):
                    lane = blk - g0
                    rhs_sb = rpool.tile([6, t_len], mat_dt, tag=f"rhs{lane}")
                    nc.sync.dma_start(rhs_sb[:], rhs_d[blk])
                    pad = 1 if not USE_V2 else 2
                    dA = dpool.tile([P, t_len + pad], row_dt, tag=f"dA{lane}")
                    dB = dpool.tile([P, t_len + pad], row_dt, tag=f"dB{lane}")
                    nc.gpsimd.memset(dA[:, pad - 1:pad], BIG)
                    nc.gpsimd.memset(dB[:, pad - 1:pad], BIG)
                    st = {"lane": lane, "blk": blk, "rhs": rhs_sb,
                          "rows": [dA, dB]}
                    lanes.append(st)

                for i in range(l_patts):
                    for st in lanes:
                        (emit_row_v2 if USE_V2 else emit_row_v1)(st, i)

                for st in lanes:
                    pad = 1 if not USE_V2 else 2
                    last_ap = st["rows"][(l_patts - 1) % 2][:, pad:t_len + pad]
                    sq = wpool.tile([P, t_len], f32, tag=f"u{st['lane']}")
                    # clamp tiny negative fp noise, then unscale
                    nc.vector.scalar_tensor_tensor(
                        sq[:], last_ap, 0.0, wj_sb[:], Alu.max, Alu.mult)
                    ot = opool.tile([P, t_len], f32, tag=f"ot{st['lane']}")
                    nc.scalar.activation(ot[:], sq[:], Act.Sqrt)
                    nc.sync.dma_start(out_d[st["blk"]], ot[:])

    nc.compile()
    return nc


def _host_prep(x, patts, w):
    """Per-core input arrays for the SPMD kernel."""
    wf = np.float64(np.float32(w))
    invw = (wf ** -np.arange(T)).astype(np.float32)          # w^-j
    if USE_V2:
        # unscale w^(j + i_last); lhsT rows carry w^-i
        wj = (wf ** (np.arange(T) + (L - 1))).astype(np.float32)
    else:
        wj = (wf ** np.arange(T)).astype(np.float32)
    wj_bcast = np.broadcast_to(wj, (P, T)).copy()

    p = np.asarray(patts, np.float32)                        # (N, L)
    lhsT = np.zeros((6, L, P), np.float32)
    for i in range(L):
        pi = p[:, i]
        si = np.float32(wf ** -i) if USE_V2 else np.float32(1.0)
        lhsT[0, i, :HALF] = pi * pi * si
        lhsT[1, i, :HALF] = -2.0 * pi * si
        lhsT[2, i, :HALF] = si
        lhsT[3, i, HALF:] = pi * pi * si
        lhsT[4, i, HALF:] = -2.0 * pi * si
        lhsT[5, i, HALF:] = si
    lhsT = lhsT.reshape(6, L * P)

    xf = np.asarray(x, np.float32)
    in_maps = []
    for c in range(NCORES):
        xs = xf[c * BLOC:(c + 1) * BLOC].reshape(NSEQ, T)    # (48, 2048)
        r1 = (xs * invw[None, :]).astype(np.float32)
        r2 = (xs * xs * invw[None, :]).astype(np.float32)
        rhs = np.empty((NBLK, 6, T), np.float32)
        rhs[:, 0] = invw
        rhs[:, 1] = r1[0::2]
        rhs[:, 2] = r2[0::2]
        rhs[:, 3] = invw
        rhs[:, 4] = r1[1::2]
        rhs[:, 5] = r2[1::2]
        in_maps.append({"rhs": rhs, "lhsT": lhsT, "wj": wj_bcast})
    return in_maps


def kernel(x, patts, w):
    from concourse.bass_utils import run_bass_kernel_spmd

    wv = float(np.float32(w))
    key = ("prog", NBLK, L, T, wv, USE_V2, MAT_F32R, INTERLEAVE)
    if key not in _cache:
        _cache[key] = _build(NBLK, L, T, wv)
    nc = _cache[key]

    in_maps = _host_prep(x, patts, w)
    res = run_bass_kernel_spmd(nc, in_maps, list(range(NCORES)))
    _cache["last_results"] = res

    outs = []
    for c in range(NCORES):
        o = res.results[c]["out"]                            # (NBLK, 128, T)
        o = o.reshape(NBLK, 2, N, T).reshape(NSEQ, N, T)     # seq-major
        o = o.reshape(BLOC, D, N, T).transpose(0, 2, 1, 3)   # (b, n, d, t)
        outs.append(o)
    return np.ascontiguousarray(np.concatenate(outs, axis=0).astype(np.float32))



# revision 8
# speedup vs baseline: 1.0265x; 1.0265x over previous
"""Trainium2 Bass kernel for DTW features (open-end weighted DTW).

Problem: x (64, 6, 2048) f32, patts (64, 32) f32, w scalar.
  c[i,j]   = (patts[n,i] - x[b,d,j])^2
  D[0,j]   = c[0,j]
  D[i,j]   = c[i,j] + w * min(D[i-1,j], D[i,j-1], D[i-1,j-1])
  out[b,n,d,j] = sqrt(D[L-1,j])

Strategy: data-parallel over batch (8 batches per core).  Per (b, n, d)
tuple the DP runs row-by-row in the scaled domain Dt[i,j] = D[i,j]*w^-j,
which turns the recurrence into a hardware min/add scan along j:

  ct[i,j]  = c[i,j] * w^-j
  u[j]     = min(w*Dt[i-1,j], Dt[i-1,j-1])          (scalar_tensor_tensor)
  Dt[i,j]  = min(u[j], Dt[i,j-1]) + ct[i,j]         (tensor_tensor_scan)
  out      = sqrt(Dt[L-1,j] * w^j)

The cost matrix ct is produced by the TensorEngine as a rank-6 matmul:
ct[(s,n), j] = [p^2, -2p, 1] . [w^-j, x_s*w^-j, x_s^2*w^-j] with two
sequences s packed per 128-partition block (64 patterns each half).
"""

import os
import sys

import numpy as np

for _p in ("/opt/trn_rl_repo", "/root/.axon_site/_ro/trn_rl_repo"):
    if _p not in sys.path and os.path.isdir(_p):
        sys.path.insert(0, _p)

B, N, D, L, T = 64, 64, 6, 32, 2048
NCORES = 8
BLOC = B // NCORES            # batches per core
NSEQ = BLOC * D               # (b, d) sequences per core
NBLK = NSEQ // 2              # two sequences per 128-partition block
P, HALF = 128, 64
BIG = 1.0e30

_cache = {}

USE_V2 = False         # bf16 state + (i+j) scaling (faster DVE min, ~2e-2 err)
MAT_F32R = False       # float32r cost matmuls (4x faster PE, ~1e-3 cost err)
INTERLEAVE = 3         # blocks emitted round-robin in groups of this size


def _build(nblk, l_patts, t_len, w):
    """Build + compile the per-core Bass program (SPMD across 8 cores)."""
    import concourse.bacc as bacc
    import concourse.bass as bass
    import concourse.mybir as mybir
    import concourse.tile as tile

    f32 = mybir.dt.float32
    Alu = mybir.AluOpType
    Act = mybir.ActivationFunctionType
    CHUNK = min(512, t_len)
    nchunk = t_len // CHUNK

    nc = bacc.Bacc("TRN2", target_bir_lowering=False, debug=False,
                   num_devices=NCORES)

    mat_dt = mybir.dt.float32r if (USE_V2 or MAT_F32R) else f32
    row_dt = mybir.dt.bfloat16 if USE_V2 else f32

    rhs_d = nc.dram_tensor("rhs", [nblk, 6, t_len], mat_dt, kind="ExternalInput")
    lhsT_d = nc.dram_tensor("lhsT", [6, l_patts * P], mat_dt, kind="ExternalInput")
    wj_d = nc.dram_tensor("wj", [P, t_len], f32, kind="ExternalInput")
    out_d = nc.dram_tensor("out", [nblk, P, t_len], f32, kind="ExternalOutput")

    with tile.TileContext(nc) as tc:
        with (
            tc.tile_pool(name="const", bufs=1) as cpool,
            tc.tile_pool(name="rhs", bufs=1) as rpool,
            tc.tile_pool(name="rows", bufs=1) as dpool,
            tc.tile_pool(name="work", bufs=2) as wpool,
            tc.tile_pool(name="outp", bufs=2) as opool,
            tc.tile_pool(name="psum", bufs=2, space=bass.MemorySpace.PSUM) as ppool,
        ):
            lhsT_sb = cpool.tile([6, l_patts * P], mat_dt)
            nc.sync.dma_start(lhsT_sb[:], lhsT_d[:])
            wj_sb = cpool.tile([P, t_len], f32)
            nc.sync.dma_start(wj_sb[:], wj_d[:])

            def emit_matmuls(ct, rhs_sb, i):
                for k in range(nchunk):
                    nc.tensor.matmul(
                        ct[:, k * CHUNK:(k + 1) * CHUNK],
                        lhsT_sb[:, i * P:(i + 1) * P],
                        rhs_sb[:, k * CHUNK:(k + 1) * CHUNK],
                        start=True, stop=True,
                    )

            def emit_row_v1(st, i):
                ct = ppool.tile([P, t_len], f32, tag="ct")
                emit_matmuls(ct, st["rhs"], i)
                cur = st["rows"][i % 2]
                if i == 0:
                    nc.scalar.activation(cur[:, 1:t_len + 1], ct[:], Act.Copy)
                else:
                    prev = st["rows"][(i - 1) % 2]
                    u = wpool.tile([P, t_len], f32, tag=f"u{st['lane']}")
                    nc.vector.scalar_tensor_tensor(
                        u[:], prev[:, 1:t_len + 1], w, prev[:, 0:t_len],
                        Alu.mult, Alu.min,
                    )
                    nc.vector.tensor_tensor_scan(
                        cur[:, 1:t_len + 1], u[:], ct[:], BIG,
                        Alu.min, Alu.add,
                    )

            def emit_row_v2(st, i):
                # row tile layout: [P, t_len+2]; col 1 = BIG boundary (the
                # j=-1 cell), data in cols 2..t_len+1 (4B-aligned for bf16).
                # u[j] = min(Dp[j], Dp[j-1]/w): the shifted scaled read is
                # done by ScalarE (Copy, scale=1/w), so the DVE min is an
                # aligned bf16 tensor_tensor at 2x.
                ct = ppool.tile([P, t_len], f32, tag="ct")
                emit_matmuls(ct, st["rhs"], i)
                cur = st["rows"][i % 2]
                if i == 0:
                    nc.scalar.activation(cur[:, 2:t_len + 2], ct[:], Act.Copy)
                else:
                    prev = st["rows"][(i - 1) % 2]
                    shl = wpool.tile([P, t_len], row_dt, tag=f"shl{st['lane']}")
                    nc.scalar.activation(shl[:], prev[:, 1:t_len + 1],
                                         Act.Copy, scale=1.0 / w)
                    u = wpool.tile([P, t_len], row_dt, tag=f"u{st['lane']}")
                    nc.vector.tensor_tensor(
                        u[:], prev[:, 2:t_len + 2], shl[:], Alu.min)
                    out_ap = (st["last"][:] if i == l_patts - 1
                              else cur[:, 2:t_len + 2])
                    nc.vector.tensor_tensor_scan(
                        out_ap, u[:], ct[:], BIG, Alu.min, Alu.add)

            G = max(1, INTERLEAVE)
            for g0 in range(0, nblk, G):
                lanes = []
                for blk in range(g0, min(g0 + G, nblk)):
                    lane = blk - g0
                    rhs_sb = rpool.tile([6, t_len], mat_dt, tag=f"rhs{lane}")
                    nc.sync.dma_start(rhs_sb[:], rhs_d[blk])
                    pad = 1 if not USE_V2 else 2
                    dA = dpool.tile([P, t_len + pad], row_dt, tag=f"dA{lane}")
                    dB = dpool.tile([P, t_len + pad], row_dt, tag=f"dB{lane}")
                    nc.gpsimd.memset(dA[:, pad - 1:pad], BIG)
                    nc.gpsimd.memset(dB[:, pad - 1:pad], BIG)
                    st = {"lane": lane, "blk": blk, "rhs": rhs_sb,
                          "rows": [dA, dB]}
                    if USE_V2:
                        st["last"] = opool.tile([P, t_len], f32,
                                                tag=f"last{lane}")
                    lanes.append(st)

                for i in range(l_patts):
                    for st in lanes:
                        (emit_row_v2 if USE_V2 else emit_row_v1)(st, i)

                for st in lanes:
                    if USE_V2:
                        last_ap = st["last"][:]
                    else:
                        last_ap = st["rows"][(l_patts - 1) % 2][:, 1:t_len + 1]
                    sq = wpool.tile([P, t_len], f32, tag=f"u{st['lane']}")
                    # clamp tiny negative fp noise, then unscale
                    nc.vector.scalar_tensor_tensor(
                        sq[:], last_ap, 0.0, wj_sb[:], Alu.max, Alu.mult)
                    ot = opool.tile([P, t_len], f32, tag=f"ot{st['lane']}")
                    nc.scalar.activation(ot[:], sq[:], Act.Sqrt)
                    nc.sync.dma_start(out_d[st["blk"]], ot[:])

    nc.compile()
    return nc


def _host_prep(x, patts, w):
    """Per-core input arrays for the SPMD kernel."""
    wf = np.float64(np.float32(w))
    invw = (wf ** -np.arange(T)).astype(np.float32)          # w^-j
    if USE_V2:
        # unscale w^(j + i_last); lhsT rows carry w^-i
        wj = (wf ** (np.arange(T) + (L - 1))).astype(np.float32)
    else:
        wj = (wf ** np.arange(T)).astype(np.float32)
    wj_bcast = np.broadcast_to(wj, (P, T)).copy()

    p = np.asarray(patts, np.float32)                        # (N, L)
    lhsT = np.zeros((6, L, P), np.float32)
    for i in range(L):
        pi = p[:, i]
        si = np.float32(wf ** -i) if USE_V2 else np.float32(1.0)
        lhsT[0, i, :HALF] = pi * pi * si
        lhsT[1, i, :HALF] = -2.0 * pi * si
        lhsT[2, i, :HALF] = si
        lhsT[3, i, HALF:] = pi * pi * si
        lhsT[4, i, HALF:] = -2.0 * pi * si
        lhsT[5, i, HALF:] = si
    lhsT = lhsT.reshape(6, L * P)

    xf = np.asarray(x, np.float32)
    in_maps = []
    for c in range(NCORES):
        xs = xf[c * BLOC:(c + 1) * BLOC].reshape(NSEQ, T)    # (48, 2048)
        r1 = (xs * invw[None, :]).astype(np.float32)
        r2 = (xs * xs * invw[None, :]).astype(np.float32)
        rhs = np.empty((NBLK, 6, T), np.float32)
        rhs[:, 0] = invw
        rhs[:, 1] = r1[0::2]
        rhs[:, 2] = r2[0::2]
        rhs[:, 3] = invw
        rhs[:, 4] = r1[1::2]
        rhs[:, 5] = r2[1::2]
        in_maps.append({"rhs": rhs, "lhsT": lhsT, "wj": wj_bcast})
    return in_maps


def kernel(x, patts, w):
    from concourse.bass_utils import run_bass_kernel_spmd

    wv = float(np.float32(w))
    key = ("prog", NBLK, L, T, wv, USE_V2, MAT_F32R, INTERLEAVE)
    if key not in _cache:
        _cache[key] = _build(NBLK, L, T, wv)
    nc = _cache[key]

    in_maps = _host_prep(x, patts, w)
    res = run_bass_kernel_spmd(nc, in_maps, list(range(NCORES)))
    _cache["last_results"] = res

    outs = []
    for c in range(NCORES):
        o = res.results[c]["out"]                            # (NBLK, 128, T)
        o = o.reshape(NBLK, 2, N, T).reshape(NSEQ, N, T)     # seq-major
        o = o.reshape(BLOC, D, N, T).transpose(0, 2, 1, 3)   # (b, n, d, t)
        outs.append(o)
    return np.ascontiguousarray(np.concatenate(outs, axis=0).astype(np.float32))

